# revision 18
# baseline (speedup 1.0000x reference)
"""Trainium2 Bass kernel for moe_routing bilinear gate.

out = sigmoid(q^T W0 r + q^T A[hop] B[hop]^T r + sum(v*q*r) + b[hop])

Key idea: fold everything hop-dependent into 5 dense matrices on the host:
    M_l = W0 + diag(v) + A[l] @ B[l]^T          (per-hop bilinear matrix)
    logit_s = q_s^T M_{hop_s} r_s + b_{hop_s}
and SORT samples by hop on the host (host prep is free). Then the device
kernel is a pure batched bilinear form with a per-512-tile constant matrix
M_l selected at compile time from the (runtime-known) hop histogram — no
masks, no gather, no hop tensor on device at all. The per-group bias is
folded into the sigmoid via the activation engine's per-partition bias
operand.

Sharding: each hop group is padded to a multiple of 8*512 and split evenly
across the 8 cores, so every core has an identical group layout and the
same compiled program runs SPMD on all cores.

Per-core pipeline over 512-sample half-tiles (feature-major bf16 layout,
[128, N'] with 16KB-contiguous chunk loads):
  - u = M_l^T q            (PE matmul, PSUM f32)
  - px = u * r             three engine paths, statically interleaved:
        A: ACT copy PSUM->SBUF bf16, then DVE mult (2x mode)
        B: DVE mult direct from PSUM
        C: Pool mult direct from PSUM
  - reduction matmul (sliding one-hot-column lhsT, K=128) accumulates 32
    tiles into one PSUM bank [32, 512]
  - sigmoid with per-partition bias b[group(tile)] (ACT), contiguous store
"""

import sys
from contextlib import ExitStack

import numpy as np
import ml_dtypes

if "/opt/trn_rl_repo" not in sys.path:
    sys.path.insert(0, "/opt/trn_rl_repo")

import concourse.bass as bass  # noqa: E402
import concourse.bacc as bacc  # noqa: E402
import concourse.tile as tile  # noqa: E402
from concourse import mybir  # noqa: E402
from concourse.bass_utils import run_bass_kernel_spmd  # noqa: E402

B_SZ, D, RHO, L = 1048576, 128, 8, 4
NG = L + 1
NCORES = 8

P = 128
TS = 512             # samples per tile (PSUM fp32 bank)
CHUNK = 8192         # samples per load DMA (2 MB per tensor)
CHUNK_BUFS = 3
FILL_TILES = 32      # tiles per output fill (PSUM [32, 512])
GRP_ALIGN = NCORES * TS  # each hop group padded to this multiple
SPLIT_FIRST = 8      # split chunk 0's load so compute starts sooner
SPLIT_LAST = 4       # split the last chunks so tail compute starts early
SPLIT_LAST_CHUNKS = 2
OUT_BF16 = True

BF16 = mybir.dt.bfloat16
F32 = mybir.dt.float32
ALU = mybir.AluOpType
ACTF = mybir.ActivationFunctionType
NPBF16 = np.dtype(ml_dtypes.bfloat16)

# px engine-path pattern (cycled statically over tiles):
#   A = ACT copy + DVE mult (2x), B = DVE mult from PSUM,
#   C = ACT copy + Pool mult (GPSIMD cannot read PSUM directly)
PX_PATTERN = "AABAACAB"

_CACHE = {}


def _emit(ctx, tc, io, sched, n):
    nc = tc.nc
    q, r, o, m5, sel, btab = io
    T = n // TS
    nf = (T + FILL_TILES - 1) // FILL_TILES
    nch = (n + CHUNK - 1) // CHUNK

    const = ctx.enter_context(tc.tile_pool(name="const", bufs=1))
    m5_s = const.tile([P, NG * P], BF16, tag="m5")
    nc.sync.dma_start(m5_s[:], m5)
    sel_s = const.tile([P, 160], BF16, tag="sel")
    nc.sync.dma_start(sel_s[:], sel)
    btab_s = const.tile([FILL_TILES, nf], F32, tag="btab")
    nc.sync.dma_start(btab_s[:], btab)

    qt_p = ctx.enter_context(tc.tile_pool(name="qt", bufs=CHUNK_BUFS))
    rt_p = ctx.enter_context(tc.tile_pool(name="rt", bufs=CHUNK_BUFS))
    us_p = ctx.enter_context(tc.tile_pool(name="us", bufs=4))
    px_p = ctx.enter_context(tc.tile_pool(name="px", bufs=4))
    fin_p = ctx.enter_context(tc.tile_pool(name="fin", bufs=2))

    up_ps = ctx.enter_context(tc.tile_pool(name="ups", bufs=3, space="PSUM"))
    out_ps = ctx.enter_context(tc.tile_pool(name="outps", bufs=2, space="PSUM"))

    chunk_tiles = {}
    up_tiles = {}
    px_tiles = {}
    acc_tiles = {}
    ov = o.rearrange("(t j) -> t j", j=TS)  # [T, 512]

    def load_chunk(ch):
        if ch < 0 or ch >= nch:
            return
        off = ch * CHUNK
        csz = min(CHUNK, n - off)
        qtl = qt_p.tile([P, CHUNK], BF16, tag="qt")
        rtl = rt_p.tile([P, CHUNK], BF16, tag="rt")
        nsplit = 1
        if ch == 0:
            # piecewise load so the first matmul starts after 1/SPLIT_FIRST
            # of the chunk has landed
            nsplit = SPLIT_FIRST
        elif ch >= nch - SPLIT_LAST_CHUNKS:
            # piecewise load so drain-tail compute overlaps the final loads
            nsplit = SPLIT_LAST
        step = max(TS, csz // nsplit)
        a = 0
        while a < csz:
            bnd = min(csz, a + step)
            nc.sync.dma_start(qtl[:, a:bnd], q[:, off + a:off + bnd])
            nc.sync.dma_start(rtl[:, a:bnd], r[:, off + a:off + bnd])
            a = bnd
        chunk_tiles[ch] = (qtl, rtl)

    def chunk_cols(t):
        ch = (t * TS) // CHUNK
        off = t * TS - ch * CHUNK
        return ch, slice(off, off + TS)

    def st_u(t):
        ch, cs = chunk_cols(t)
        qtl = chunk_tiles[ch][0]
        l = sched[t]
        up = up_ps.tile([P, TS], F32, tag="up")
        nc.tensor.matmul(up[:], m5_s[:, l * P:(l + 1) * P], qtl[:, cs],
                         start=True, stop=True)
        up_tiles[t] = up

    def st_px(t):
        if t < 0 or t >= T:
            return
        ch, cs = chunk_cols(t)
        rtl = chunk_tiles[ch][1]
        up = up_tiles.pop(t)
        px = px_p.tile([P, TS], BF16, tag="px")
        path = PX_PATTERN[t % len(PX_PATTERN)]
        if path == "A":
            us = us_p.tile([P, TS], BF16, tag="us")
            nc.scalar.copy(us[:], up[:])
            nc.vector.tensor_tensor(px[:], us[:], rtl[:, cs], ALU.mult)
        elif path == "B":
            nc.vector.tensor_tensor(px[:], up[:], rtl[:, cs], ALU.mult)
        else:
            us = us_p.tile([P, TS], BF16, tag="us")
            nc.scalar.copy(us[:], up[:])
            nc.gpsimd.tensor_tensor(px[:], us[:], rtl[:, cs], ALU.mult)
        px_tiles[t] = px

    def st_red(t):
        if t < 0 or t >= T:
            return
        f, tt = t // FILL_TILES, t % FILL_TILES
        ntf = min(FILL_TILES, T - f * FILL_TILES)
        if tt == 0:
            acc_tiles[f] = out_ps.tile([P, TS], F32, tag="out",
                                       name="out_acc")
        nc.tensor.matmul(
            acc_tiles[f][:], sel_s[:, 31 - tt:31 - tt + P],
            px_tiles.pop(t)[:],
            start=(tt == 0), stop=(tt == ntf - 1),
            skip_group_check=True,
        )
        if tt == ntf - 1:
            st_fin(f, ntf)

    def st_fin(f, ntf):
        out_acc = acc_tiles.pop(f)
        osb = fin_p.tile([FILL_TILES, TS], BF16 if OUT_BF16 else F32,
                         tag="osb")
        nc.scalar.activation(
            osb[0:ntf, :], out_acc[0:ntf, :], ACTF.Sigmoid,
            bias=btab_s[0:ntf, f:f + 1], scale=1.0,
        )
        nc.scalar.dma_start(ov[f * FILL_TILES:f * FILL_TILES + ntf, :],
                            osb[0:ntf, :])

    halves_per_chunk = CHUNK // TS
    load_chunk(0)
    load_chunk(1)
    for t in range(T + 2):
        if t < T:
            if t % halves_per_chunk == 0:
                load_chunk(t // halves_per_chunk + 1)
            st_u(t)
        st_px(t - 1)
        st_red(t - 2)


def _build(nl_key):
    if nl_key in _CACHE:
        return _CACHE[nl_key]
    n = sum(nl_key)
    T = n // TS
    sched = []
    for l, nl in enumerate(nl_key):
        sched.extend([l] * (nl // TS))
    assert len(sched) == T
    nf = (T + FILL_TILES - 1) // FILL_TILES

    nc = bacc.Bacc("TRN2", target_bir_lowering=False, debug=False)
    q = nc.dram_tensor("q", [P, n], BF16, kind="ExternalInput").ap()
    r = nc.dram_tensor("r", [P, n], BF16, kind="ExternalInput").ap()
    o = nc.dram_tensor("o", [n], BF16 if OUT_BF16 else F32,
                       kind="ExternalOutput").ap()
    m5 = nc.dram_tensor("m5", [P, NG * P], BF16, kind="ExternalInput").ap()
    sel = nc.dram_tensor("sel", [P, 160], BF16, kind="ExternalInput").ap()
    btab = nc.dram_tensor("btab", [FILL_TILES, nf], F32,
                          kind="ExternalInput").ap()
    io = (q, r, o, m5, sel, btab)
    with tile.TileContext(nc) as tc, ExitStack() as ctx:
        _emit(ctx, tc, io, sched, n)
    nc.compile()
    _CACHE[nl_key] = nc
    return nc


def _prep(q, r, hop, W0, A, Bm, v, b):
    q = np.asarray(q, dtype=np.float32)
    r = np.asarray(r, dtype=np.float32)
    hop = np.asarray(hop).astype(np.int64)
    W0 = np.asarray(W0, dtype=np.float32)
    A = np.asarray(A, dtype=np.float32)
    Bm = np.asarray(Bm, dtype=np.float32)
    v = np.asarray(v, dtype=np.float32)
    b = np.asarray(b, dtype=np.float32)

    M = np.stack([W0[0] + np.diag(v) + A[l] @ Bm[l].T for l in range(NG)])
    # [128 d, 5*128] with cols l*128+e; lhsT slice l gives u = M_l^T q
    m5 = np.ascontiguousarray(
        M.transpose(1, 0, 2).reshape(D, NG * D)).astype(NPBF16)
    sel = np.zeros((P, 160), dtype=NPBF16)
    sel[:, 31] = 1.0

    counts = np.bincount(hop, minlength=NG)
    order = np.argsort(hop, kind="stable")
    # per-core per-group counts, multiples of TS; identical across cores
    n_l = [int(-(-c // GRP_ALIGN) * GRP_ALIGN) // NCORES for c in counts]
    nl_key = tuple(n_l)
    n = sum(n_l)
    T = n // TS
    nf = (T + FILL_TILES - 1) // FILL_TILES

    sched = []
    for l, nl in enumerate(n_l):
        sched.extend([l] * (nl // TS))
    btab = np.zeros((FILL_TILES, nf), dtype=np.float32)
    for t in range(T):
        btab[t % FILL_TILES, t // FILL_TILES] = b[sched[t]]

    # sample index array per core: sentinel B_SZ -> zero row (padding)
    core_idx = np.empty((NCORES, n), dtype=np.int64)
    start = 0
    for l in range(NG):
        idx = order[start:start + counts[l]]
        start += counts[l]
        pad = n_l[l] * NCORES - counts[l]
        idxp = np.concatenate([idx, np.full(pad, B_SZ, dtype=np.int64)])
        off = sum(n_l[:l])
        core_idx[:, off:off + n_l[l]] = idxp.reshape(NCORES, n_l[l])

    q1 = np.concatenate([q, np.zeros((1, D), np.float32)], axis=0)
    r1 = np.concatenate([r, np.zeros((1, D), np.float32)], axis=0)

    consts = dict(m5=m5, sel=sel, btab=btab)
    in_maps = []
    for c in range(NCORES):
        ci = core_idx[c]
        qt = np.ascontiguousarray(q1[ci].T).astype(NPBF16)
        rt = np.ascontiguousarray(r1[ci].T).astype(NPBF16)
        in_maps.append(dict(q=qt, r=rt, **consts))
    return in_maps, nl_key, core_idx


def _run(inputs, trace=False, tmpdir=None):
    in_maps, nl_key, core_idx = _prep(**inputs)
    nc = _build(nl_key)
    res = run_bass_kernel_spmd(
        nc, in_maps, list(range(NCORES)), trace=trace, tmpdir=tmpdir
    )
    out = np.empty(B_SZ, dtype=np.float32)
    for c in range(NCORES):
        oc = np.asarray(res.results[c]["o"]).astype(np.float32)
        ci = core_idx[c]
        valid = ci < B_SZ
        out[ci[valid]] = oc[valid]
    return out, res


def kernel(**inputs):
    out, _ = _run(inputs)
    return out


# revision 27
# speedup vs baseline: 1.3577x; 1.3577x over previous
"""Trainium2 Bass kernel for moe_routing bilinear gate.

out = sigmoid(q^T W0 r + q^T A[hop] B[hop]^T r + sum(v*q*r) + b[hop])

Key idea: fold everything hop-dependent into 5 dense matrices on the host:
    M_l = W0 + diag(v) + A[l] @ B[l]^T          (per-hop bilinear matrix)
    logit_s = q_s^T M_{hop_s} r_s + b_{hop_s}
and SORT samples by hop on the host (host prep is free). Then the device
kernel is a pure batched bilinear form with a per-512-tile constant matrix
M_l selected at compile time from the (runtime-known) hop histogram — no
masks, no gather, no hop tensor on device at all. The per-group bias is
folded into the sigmoid via the activation engine's per-partition bias
operand.

Sharding: each hop group is padded to a multiple of 8*512 and split evenly
across the 8 cores, so every core has an identical group layout and the
same compiled program runs SPMD on all cores.

Per-core pipeline over 512-sample half-tiles (feature-major bf16 layout,
[128, N'] with 16KB-contiguous chunk loads):
  - u = M_l^T q            (PE matmul, PSUM f32)
  - px = u * r             three engine paths, statically interleaved:
        A: ACT copy PSUM->SBUF bf16, then DVE mult (2x mode)
        B: DVE mult direct from PSUM
        C: Pool mult direct from PSUM
  - reduction matmul (sliding one-hot-column lhsT, K=128) accumulates 32
    tiles into one PSUM bank [32, 512]
  - sigmoid with per-partition bias b[group(tile)] (ACT), contiguous store
"""

import sys
from contextlib import ExitStack

import numpy as np
import ml_dtypes

if "/opt/trn_rl_repo" not in sys.path:
    sys.path.insert(0, "/opt/trn_rl_repo")

import concourse.bass as bass  # noqa: E402
import concourse.bacc as bacc  # noqa: E402
import concourse.tile as tile  # noqa: E402
from concourse import mybir  # noqa: E402
from concourse.bass_utils import run_bass_kernel_spmd  # noqa: E402

B_SZ, D, RHO, L = 1048576, 128, 8, 4
NG = L + 1
NCORES = 8

P = 128
TS = 512             # samples per tile (PSUM fp32 bank)
CHUNK = 8192         # samples per load DMA (2 MB per tensor)
CHUNK_BUFS = 3
FILL_TILES = 32      # tiles per output fill (PSUM [32, 512])
GRP_ALIGN = NCORES * TS  # each hop group padded to this multiple
SPLIT_FIRST = 8      # split chunk 0's load so compute starts sooner
SPLIT_LAST = 4       # split the last chunks so tail compute starts early
SPLIT_LAST_CHUNKS = 2
OUT_BF16 = True

BF16 = mybir.dt.bfloat16
F32 = mybir.dt.float32
ALU = mybir.AluOpType
ACTF = mybir.ActivationFunctionType
NPBF16 = np.dtype(ml_dtypes.bfloat16)

# px engine-path pattern (cycled statically over tiles):
#   A = ACT copy + DVE mult (2x), B = DVE mult from PSUM,
#   C = ACT copy + Pool mult (GPSIMD cannot read PSUM directly)
PX_PATTERN = "AABAACAB"

_CACHE = {}


def _fills(T):
    """Fill sizes (tiles per output PSUM block). Mostly 32; the tail is
    rebalanced so the penultimate fill finishes before the drain tail."""
    fills = []
    rem = T
    while rem > 0:
        take = min(FILL_TILES, rem)
        fills.append(take)
        rem -= take
    return fills


def _emit(ctx, tc, io, sched, n):
    nc = tc.nc
    q, r, o, m5, sel, btab = io
    T = n // TS
    fills = _fills(T)
    nf = len(fills)
    tile_f, tile_tt, fill_off = [], [], []
    t0 = 0
    for f, sz in enumerate(fills):
        fill_off.append(t0)
        tile_f.extend([f] * sz)
        tile_tt.extend(range(sz))
        t0 += sz
    nch = (n + CHUNK - 1) // CHUNK

    const = ctx.enter_context(tc.tile_pool(name="const", bufs=1))
    m5_s = const.tile([P, NG * P], BF16, tag="m5")
    nc.sync.dma_start(m5_s[:], m5)
    sel_s = const.tile([P, 160], BF16, tag="sel")
    nc.sync.dma_start(sel_s[:], sel)
    btab_s = const.tile([FILL_TILES, nf], F32, tag="btab")
    nc.sync.dma_start(btab_s[:], btab)

    qt_p = ctx.enter_context(tc.tile_pool(name="qt", bufs=CHUNK_BUFS))
    rt_p = ctx.enter_context(tc.tile_pool(name="rt", bufs=CHUNK_BUFS))
    us_p = ctx.enter_context(tc.tile_pool(name="us", bufs=4))
    px_p = ctx.enter_context(tc.tile_pool(name="px", bufs=4))
    fin_p = ctx.enter_context(tc.tile_pool(name="fin", bufs=2))

    up_ps = ctx.enter_context(tc.tile_pool(name="ups", bufs=3, space="PSUM"))
    out_ps = ctx.enter_context(tc.tile_pool(name="outps", bufs=2, space="PSUM"))

    chunk_tiles = {}
    up_tiles = {}
    px_tiles = {}
    acc_tiles = {}
    ov = o.rearrange("(t j) -> t j", j=TS)  # [T, 512]

    def load_chunk(ch):
        if ch < 0 or ch >= nch:
            return
        off = ch * CHUNK
        csz = min(CHUNK, n - off)
        qtl = qt_p.tile([P, CHUNK], BF16, tag="qt")
        rtl = rt_p.tile([P, CHUNK], BF16, tag="rt")
        nsplit = 1
        if ch == 0:
            # piecewise load so the first matmul starts after 1/SPLIT_FIRST
            # of the chunk has landed
            nsplit = SPLIT_FIRST
        elif ch >= nch - SPLIT_LAST_CHUNKS:
            # piecewise load so drain-tail compute overlaps the final loads
            nsplit = SPLIT_LAST
        step = max(TS, csz // nsplit)
        a = 0
        while a < csz:
            bnd = min(csz, a + step)
            nc.sync.dma_start(qtl[:, a:bnd], q[:, off + a:off + bnd])
            nc.sync.dma_start(rtl[:, a:bnd], r[:, off + a:off + bnd])
            a = bnd
        chunk_tiles[ch] = (qtl, rtl)

    def chunk_cols(t):
        ch = (t * TS) // CHUNK
        off = t * TS - ch * CHUNK
        return ch, slice(off, off + TS)

    def st_u(t):
        ch, cs = chunk_cols(t)
        qtl = chunk_tiles[ch][0]
        l = sched[t]
        up = up_ps.tile([P, TS], F32, tag="up")
        nc.tensor.matmul(up[:], m5_s[:, l * P:(l + 1) * P], qtl[:, cs],
                         start=True, stop=True)
        up_tiles[t] = up

    def st_px(t):
        if t < 0 or t >= T:
            return
        ch, cs = chunk_cols(t)
        rtl = chunk_tiles[ch][1]
        up = up_tiles.pop(t)
        px = px_p.tile([P, TS], BF16, tag="px")
        if t >= T - 16:
            # drain tail: alternate the two fastest paths so the final
            # tiles clear the vector engines as quickly as possible
            path = "AAB"[t % 3]
        else:
            path = PX_PATTERN[t % len(PX_PATTERN)]
        if path == "A":
            us = us_p.tile([P, TS], BF16, tag="us")
            nc.scalar.copy(us[:], up[:])
            nc.vector.tensor_tensor(px[:], us[:], rtl[:, cs], ALU.mult)
        elif path == "B":
            nc.vector.tensor_tensor(px[:], up[:], rtl[:, cs], ALU.mult)
        else:
            us = us_p.tile([P, TS], BF16, tag="us")
            nc.scalar.copy(us[:], up[:])
            nc.gpsimd.tensor_tensor(px[:], us[:], rtl[:, cs], ALU.mult)
        px_tiles[t] = px

    def st_red(t):
        if t < 0 or t >= T:
            return
        f, tt = tile_f[t], tile_tt[t]
        ntf = fills[f]
        if tt == 0:
            acc_tiles[f] = out_ps.tile([P, TS], F32, tag="out",
                                       name="out_acc")
        nc.tensor.matmul(
            acc_tiles[f][:], sel_s[:, 31 - tt:31 - tt + P],
            px_tiles.pop(t)[:],
            start=(tt == 0), stop=(tt == ntf - 1),
            skip_group_check=True,
        )
        if tt == ntf - 1:
            st_fin(f, ntf)

    def st_fin(f, ntf):
        out_acc = acc_tiles.pop(f)
        osb = fin_p.tile([FILL_TILES, TS], BF16 if OUT_BF16 else F32,
                         tag="osb")
        nc.scalar.activation(
            osb[0:ntf, :], out_acc[0:ntf, :], ACTF.Sigmoid,
            bias=btab_s[0:ntf, f:f + 1], scale=1.0,
        )
        nc.scalar.dma_start(ov[fill_off[f]:fill_off[f] + ntf, :],
                            osb[0:ntf, :])

    halves_per_chunk = CHUNK // TS
    load_chunk(0)
    load_chunk(1)
    for t in range(T + 2):
        if t < T:
            if t % halves_per_chunk == 0 and t > 0:
                load_chunk(t // halves_per_chunk + 1)
            st_u(t)
        st_px(t - 1)
        st_red(t - 2)


def _build(nl_key):
    if nl_key in _CACHE:
        return _CACHE[nl_key]
    n = sum(nl_key)
    T = n // TS
    sched = []
    for l, nl in enumerate(nl_key):
        sched.extend([l] * (nl // TS))
    assert len(sched) == T
    nf = len(_fills(T))

    nc = bacc.Bacc("TRN2", target_bir_lowering=False, debug=False)
    q = nc.dram_tensor("q", [P, n], BF16, kind="ExternalInput").ap()
    r = nc.dram_tensor("r", [P, n], BF16, kind="ExternalInput").ap()
    o = nc.dram_tensor("o", [n], BF16 if OUT_BF16 else F32,
                       kind="ExternalOutput").ap()
    m5 = nc.dram_tensor("m5", [P, NG * P], BF16, kind="ExternalInput").ap()
    sel = nc.dram_tensor("sel", [P, 160], BF16, kind="ExternalInput").ap()
    btab = nc.dram_tensor("btab", [FILL_TILES, nf], F32,
                          kind="ExternalInput").ap()
    io = (q, r, o, m5, sel, btab)
    with tile.TileContext(nc) as tc, ExitStack() as ctx:
        _emit(ctx, tc, io, sched, n)
    nc.compile()
    _CACHE[nl_key] = nc
    return nc


def _prep(q, r, hop, W0, A, Bm, v, b):
    q = np.asarray(q, dtype=np.float32)
    r = np.asarray(r, dtype=np.float32)
    hop = np.asarray(hop).astype(np.int64)
    W0 = np.asarray(W0, dtype=np.float32)
    A = np.asarray(A, dtype=np.float32)
    Bm = np.asarray(Bm, dtype=np.float32)
    v = np.asarray(v, dtype=np.float32)
    b = np.asarray(b, dtype=np.float32)

    M = np.stack([W0[0] + np.diag(v) + A[l] @ Bm[l].T for l in range(NG)])
    # [128 d, 5*128] with cols l*128+e; lhsT slice l gives u = M_l^T q
    m5 = np.ascontiguousarray(
        M.transpose(1, 0, 2).reshape(D, NG * D)).astype(NPBF16)
    sel = np.zeros((P, 160), dtype=NPBF16)
    sel[:, 31] = 1.0

    counts = np.bincount(hop, minlength=NG)
    order = np.argsort(hop, kind="stable")
    # per-core per-group counts, multiples of TS; identical across cores
    n_l = [int(-(-c // GRP_ALIGN) * GRP_ALIGN) // NCORES for c in counts]
    nl_key = tuple(n_l)
    n = sum(n_l)
    T = n // TS

    sched = []
    for l, nl in enumerate(n_l):
        sched.extend([l] * (nl // TS))
    fills = _fills(T)
    nf = len(fills)
    btab = np.zeros((FILL_TILES, nf), dtype=np.float32)
    t0 = 0
    for f, sz in enumerate(fills):
        for tt in range(sz):
            btab[tt, f] = b[sched[t0 + tt]]
        t0 += sz

    # sample index array per core: sentinel B_SZ -> zero row (padding)
    core_idx = np.empty((NCORES, n), dtype=np.int64)
    start = 0
    for l in range(NG):
        idx = order[start:start + counts[l]]
        start += counts[l]
        pad = n_l[l] * NCORES - counts[l]
        idxp = np.concatenate([idx, np.full(pad, B_SZ, dtype=np.int64)])
        off = sum(n_l[:l])
        core_idx[:, off:off + n_l[l]] = idxp.reshape(NCORES, n_l[l])

    q1 = np.concatenate([q, np.zeros((1, D), np.float32)], axis=0)
    r1 = np.concatenate([r, np.zeros((1, D), np.float32)], axis=0)

    consts = dict(m5=m5, sel=sel, btab=btab)
    in_maps = []
    for c in range(NCORES):
        ci = core_idx[c]
        qt = np.ascontiguousarray(q1[ci].T).astype(NPBF16)
        rt = np.ascontiguousarray(r1[ci].T).astype(NPBF16)
        in_maps.append(dict(q=qt, r=rt, **consts))
    return in_maps, nl_key, core_idx


def _run(inputs, trace=False, tmpdir=None):
    in_maps, nl_key, core_idx = _prep(**inputs)
    nc = _build(nl_key)
    res = run_bass_kernel_spmd(
        nc, in_maps, list(range(NCORES)), trace=trace, tmpdir=tmpdir
    )
    out = np.empty(B_SZ, dtype=np.float32)
    for c in range(NCORES):
        oc = np.asarray(res.results[c]["o"]).astype(np.float32)
        ci = core_idx[c]
        valid = ci < B_SZ
        out[ci[valid]] = oc[valid]
    return out, res


def kernel(**inputs):
    out, _ = _run(inputs)
    return out
